# revision 10
# baseline (speedup 1.0000x reference)
"""Contrastive-loss kernel for 8 TRN2 NeuronCores (Bass/Tile) — v7 screened scan.

Math: loss = (P + Q) / (n(n-1)).  P exact via per-class sums (device matmul,
host finish).  Q == 0 certified on-device by scanning every pair that is not
provably margin-safe, with t = -(d2 + 256*same_mask) from one bf16 augmented
matmul; flag iff t > -THETA; host computes exact Q iff flagged.

v7 projection screen: for any unit u, d2_ij >= (u.x_i - u.x_j)^2, so a pair
with |s_i - s_j| >= 1 (s = u.x, u = top right-singular direction, fp64 host
math) can never violate the margin and needs no scan.  Sorting points by s
makes each 128-row tile's eligible columns one contiguous range [t*128+1, hi)
in sorted order (upper-triangle assignment; within-tile lower dups are
harmless).  This covers every pair with |ds| < 1 exactly once or more and
cuts the scanned stream from 33792 to ~14.5k columns per core (~0.43x).

SPMD layout: 64 row-tiles sorted by eligible width, rank-octile k forms slot
k with shared padded width W[k]; core c takes one tile per slot, so all
cores run the identical instruction stream.  Padding repeats real columns.

Scan pipeline (from v5): stream cut into 1024-col jobs, one [128,1024] f32
PSUM tile each (pool bufs=4 = all 8 banks).  ACT jobs chain in place with
out = Relu(C*v + C*THETA) (residue amplified each lap; next job on the slot
accumulates via start=False matmuls).  DVE jobs extract flags directly via
reduce_max.  Tail jobs (last on each slot) extract: ACT via Relu+accum_out,
DVE via reduce_max.  Engine split 8 ACT / 7 DVE jobs balances the 1.2 GHz
ACT vs 0.96 GHz DVE drain rates.  Per-bank start=True rule: only the first
matmul segment touching a PSUM bank clears it; later segments in the same
bank use start=False (bank-wide has_written clear would wipe earlier
segments' accumulate bits).
"""

import numpy as np
import ml_dtypes

import concourse.bass as bass
import concourse.bacc as bacc
import concourse.tile as tile
from concourse import mybir
from concourse.bass_utils import run_bass_kernel_spmd

MARGIN = 1.0

N, D, NCLS, CORES = 8192, 64, 8, 8
PT = 128                     # rows per row-tile (partition dim)
NTILE = N // PT              # 64 row-tiles
NSLOT_T = NTILE // CORES     # 8 tile-slots per core
JOB = 1024                   # columns per consumer job
NSLOT = 4                    # PSUM pool bufs
KAUG = D + 2 + NCLS          # 74
MSCALE = 16.0                # onehot scale; same-label mask adds 256
THETA = 2.0                  # flag threshold on d2
CAMP = 4096.0                # chain amplification
NCHUNK = NSLOT_T             # class-sum K chunks (8 x 128 rows per core)
FDIM = D + 2                 # [x | sq | 1]

# default slot widths for the canonical setup_inputs() data (recomputed at
# runtime from the actual inputs; must be multiples of 8)
W_DEFAULT = (2432, 2360, 2224, 2072, 1848, 1600, 1208, 720)


def _schedule(widths):
    """Static per-job tables for slot widths `widths` (len 8, mult of 8)."""
    W = int(sum(widths))
    njob = -(-W // JOB)
    sizes = [min(JOB, W - JOB * j) for j in range(njob)]
    # job j is the last on its PSUM slot (or its successor is too short to
    # cover j's full chain-residue region) -> must extract a flag.
    tail = []
    for j in range(njob):
        nxt = j + NSLOT
        tail.append(nxt > njob - 1 or sizes[nxt] < sizes[j])
    # Engine per job.  Tail jobs always go to DVE: reduce_max both consumes
    # the tile and extracts the flag in one 1x pass, while an ACT tail needs
    # the pricey accum_out accumulator read.  ACT handles a prefix of the
    # non-tail jobs as pure in-place relu chains; prefix size balances
    # ACT @(fd+172)/1.2 ns against DVE @(fd+120)/0.96 ns totals.
    nontail = [j for j in range(njob) if not tail[j]]
    t_tail = sum((sizes[j] + 120) / 0.96 for j in range(njob) if tail[j])
    best_na, best_t = 0, float("inf")
    for na in range(len(nontail) + 1):
        t_a = sum((sizes[j] + 172) / 1.2 for j in nontail[:na])
        t_d = t_tail + sum((sizes[j] + 120) / 0.96 for j in nontail[na:])
        if max(t_a, t_d) < best_t:
            best_na, best_t = na, max(t_a, t_d)
    a_set = set(nontail[:best_na])
    eng = ["A" if j in a_set else "D" for j in range(njob)]
    accum = [j >= NSLOT and eng[j - NSLOT] == "A" and not tail[j - NSLOT]
             for j in range(njob)]
    extract_col = {}
    for j in range(njob):
        if eng[j] == "D" or tail[j]:
            extract_col[j] = len(extract_col)
    # matmul segments per job: cut at slot boundaries + 512 PSUM-bank grid
    offs = np.cumsum([0] + list(widths))
    segs = []
    for j in range(njob):
        a, b = JOB * j, JOB * j + sizes[j]
        cuts = {a, b}
        cuts.update(int(o) for o in offs if a < o < b)
        cuts.update(a + k * 512 for k in range(1, -(-sizes[j] // 512)))
        cuts = sorted(c for c in cuts if a <= c <= b)
        sj = []
        for u, v in zip(cuts, cuts[1:]):
            g = int(np.searchsorted(offs, u, side="right")) - 1  # slot idx
            sj.append((u - a, v - u, g, u))   # (tile_off, width, slot, rhs col)
        segs.append(sj)
    return {"W": W, "njob": njob, "sizes": sizes, "eng": eng, "tail": tail,
            "accum": accum, "extract_col": extract_col, "segs": segs,
            "nviol": len(extract_col), "widths": tuple(int(w) for w in widths)}


def build_nc(repeats: int = 1, widths=W_DEFAULT, loop: int | None = None):
    """Build the SPMD kernel.  `repeats` unrolled reps; if `loop` is given,
    a hardware For_i loop runs the unrolled body `loop` times (total reps =
    repeats * loop) — used for precise wall-clock timing where per-dispatch
    overhead would otherwise swamp the signal."""
    sch = _schedule(widths)
    nc = bacc.Bacc("TRN2", target_bir_lowering=False, debug=False,
                   num_devices=CORES)
    bf16, f32 = mybir.dt.bfloat16, mybir.dt.float32
    W, njob = sch["W"], sch["njob"]

    lhst_d = nc.dram_tensor("lhst", [KAUG, NSLOT_T * PT], bf16,
                            kind="ExternalInput")
    rhs_d = nc.dram_tensor("rhs", [KAUG, W], bf16, kind="ExternalInput")
    clsoh_d = nc.dram_tensor("clsoh", [PT, NCHUNK, NCLS], f32,
                             kind="ExternalInput")
    clsft_d = nc.dram_tensor("clsft", [PT, NCHUNK, FDIM], f32,
                             kind="ExternalInput")
    viol_d = nc.dram_tensor("viol", [PT, sch["nviol"]], f32,
                            kind="ExternalOutput")
    cls_d = nc.dram_tensor("cls", [NCLS, FDIM], f32, kind="ExternalOutput")

    with tile.TileContext(nc) as tc:
        with (
            tc.tile_pool(name="w", bufs=1) as wpool,
            tc.tile_pool(name="ps", bufs=NSLOT, space="PSUM") as pspool,
            tc.tile_pool(name="acc", bufs=1) as accpool,
        ):
            lhst = wpool.tile([KAUG, NSLOT_T * PT], bf16)
            nc.sync.dma_start(out=lhst[:], in_=lhst_d[:])
            rhs = wpool.tile([KAUG, W], bf16)
            for a in range(0, W, 2048):
                b = min(a + 2048, W)
                nc.sync.dma_start(out=rhs[:, a:b], in_=rhs_d[:, a:b])
            clsoh = wpool.tile([PT, NCHUNK, NCLS], f32)
            nc.sync.dma_start(out=clsoh[:], in_=clsoh_d[:])
            clsft = wpool.tile([PT, NCHUNK, FDIM], f32)
            nc.sync.dma_start(out=clsft[:], in_=clsft_d[:])

            viol_sb = accpool.tile([PT, sch["nviol"]], f32)
            cls_sb = accpool.tile([NCLS, FDIM], f32)
            theta_sb = accpool.tile([PT, 1], f32)
            nc.vector.memset(theta_sb, THETA)
            biasc_sb = accpool.tile([PT, 1], f32)
            nc.vector.memset(biasc_sb, CAMP * THETA)

            def rep_body():
              for _rep in range(repeats):
                for j in range(njob):
                    fd = sch["sizes"][j]
                    ps = pspool.tile([PT, JOB], f32, tag="ps")
                    st_job = not sch["accum"][j]
                    seen_banks = set()
                    for off, width, g, col in sch["segs"][j]:
                        bank = off // 512
                        st = st_job and bank not in seen_banks
                        seen_banks.add(bank)
                        nc.tensor.matmul(
                            ps[:, off:off + width],
                            lhst[:, g * PT:(g + 1) * PT],
                            rhs[:, col:col + width],
                            start=st, stop=True)
                    ecol = sch["extract_col"].get(j)
                    if sch["eng"][j] == "A":
                        if sch["tail"][j]:   # final lap: extract the flag
                            nc.scalar.activation(
                                out=ps[:, :fd], in_=ps[:, :fd],
                                func=mybir.ActivationFunctionType.Relu,
                                bias=theta_sb[:], scale=1.0,
                                accum_out=viol_sb[:, ecol:ecol + 1])
                        else:                # chain: C*relu(v+THETA) in place
                            nc.scalar.activation(
                                out=ps[:, :fd], in_=ps[:, :fd],
                                func=mybir.ActivationFunctionType.Relu,
                                bias=biasc_sb[:], scale=CAMP)
                    else:
                        nc.vector.tensor_reduce(
                            out=viol_sb[:, ecol:ecol + 1],
                            in_=ps[:, :fd],
                            axis=mybir.AxisListType.X,
                            op=mybir.AluOpType.max)

                # class sums (fp32) in a tag-shared slot
                psc = pspool.tile([PT, JOB], f32, tag="ps")
                for i in range(NCHUNK):
                    nc.tensor.matmul(
                        psc[:NCLS, 0:FDIM],
                        clsoh[:, i, :],
                        clsft[:, i, :],
                        start=(i == 0), stop=(i == NCHUNK - 1))
                nc.scalar.copy(out=cls_sb[:], in_=psc[:NCLS, 0:FDIM])

            if loop is not None:
                with tc.For_i(0, loop):
                    rep_body()
            else:
                rep_body()

            nc.sync.dma_start(out=viol_d[:], in_=viol_sb[:])
            nc.sync.dma_start(out=cls_d[:], in_=cls_sb[:])
    nc.compile()
    return nc


def _screen(x64: np.ndarray):
    """Projection screen: sorted order, per-tile eligible widths, slot
    assignment.  Returns (order, widths[NTILE], W[8], tile_of[core][slot])."""
    G = x64.T @ x64
    _, V = np.linalg.eigh(G)
    u = V[:, -1]
    u = u / np.linalg.norm(u) * (1 - 1e-12)
    s = x64 @ u
    order = np.argsort(s, kind="stable")
    ss = s[order]
    widths = np.empty(NTILE, np.int64)
    for g in range(NTILE):
        hi = np.searchsorted(ss, ss[PT * g + PT - 1] + MARGIN + 1e-9)
        widths[g] = hi - (PT * g + 1)
    rank = np.argsort(-widths, kind="stable")
    W = []
    tile_of = [[0] * NSLOT_T for _ in range(CORES)]
    for k in range(NSLOT_T):
        grp = rank[CORES * k:CORES * k + CORES]
        W.append(int(np.ceil(widths[grp].max() / 8) * 8))
        for c in range(CORES):
            tile_of[c][k] = int(grp[c])
    return order, widths, tuple(W), tile_of


def prep_inputs(x: np.ndarray, label: np.ndarray, screen=None):
    """Host-side prep: screen + per-core augmented bf16 operands."""
    x64 = x.astype(np.float64)
    order, widths, W, tile_of = screen if screen is not None else _screen(x64)
    sq = (x64 * x64).sum(axis=1)
    oh = np.zeros((N, NCLS), np.float64)
    oh[np.arange(N), label] = 1.0

    lhst_all = np.concatenate(
        [x64, sq[:, None], np.ones((N, 1)), MSCALE * oh], axis=1
    ).T.astype(ml_dtypes.bfloat16)                     # [KAUG, N]
    rhs_all = np.concatenate(
        [2.0 * x64, -np.ones((N, 1)), -sq[:, None], -MSCALE * oh], axis=1
    ).T.astype(ml_dtypes.bfloat16)                     # [KAUG, N]

    feat = np.concatenate([x64, sq[:, None], np.ones((N, 1))], axis=1
                          ).astype(np.float32)         # [N, FDIM]
    ohf = oh.astype(np.float32)                        # [N, NCLS]

    in_maps = []
    for cc in range(CORES):
        rows = np.concatenate(
            [order[g * PT:(g + 1) * PT] for g in tile_of[cc]])
        cols = []
        for k, g in enumerate(tile_of[cc]):
            w = int(widths[g])
            idx = order[g * PT + 1: g * PT + 1 + w]
            if w < W[k]:
                idx = np.concatenate([idx, np.full(W[k] - w, idx[-1])])
            cols.append(idx)
        cols = np.concatenate(cols)
        in_maps.append({
            "lhst": np.ascontiguousarray(lhst_all[:, rows]),
            "rhs": np.ascontiguousarray(rhs_all[:, cols]),
            "clsoh": np.ascontiguousarray(
                ohf[rows].reshape(NCHUNK, PT, NCLS).transpose(1, 0, 2)),
            "clsft": np.ascontiguousarray(
                feat[rows].reshape(NCHUNK, PT, FDIM).transpose(1, 0, 2)),
        })
    return in_maps


def _exact_q(x: np.ndarray, label: np.ndarray) -> float:
    """Exact Q = sum over ordered diff-label pairs of relu(1-d)^2 (fp64,
    chunked).  Only runs when the device flags a potential margin pair."""
    x64 = x.astype(np.float64)
    sq = (x64 * x64).sum(axis=1)
    q = 0.0
    step = 1024
    for a in range(0, N, step):
        d2 = sq[a:a + step, None] + sq[None, :] - 2.0 * (x64[a:a + step] @ x64.T)
        d = np.sqrt(np.maximum(d2, 0.0))
        diff = label[a:a + step, None] != label[None, :]
        r = np.maximum(MARGIN - d, 0.0)
        offdiag = np.arange(a, a + step)[:, None] != np.arange(N)[None, :]
        q += float((r * r)[diff & offdiag].sum())
    return q


def finish(results, sch, x: np.ndarray, label: np.ndarray) -> np.float32:
    cls = np.zeros((NCLS, FDIM), np.float64)
    for rr in results:
        cls += rr["cls"].astype(np.float64)
    M = cls[:, :D]
    S = cls[:, D]
    ncnt = cls[:, D + 1]
    P = float((2.0 * ncnt * S - 2.0 * (M * M).sum(axis=1)).sum())

    flagged = False
    for rr in results:
        v = rr["viol"]
        if not np.isfinite(v).all():                  # chain overflow
            flagged = True
            continue
        for j, col in sch["extract_col"].items():
            if sch["eng"][j] == "A":
                if (v[:, col] > 0.0).any():           # tail relu-accum
                    flagged = True
            else:
                if (v[:, col] > -THETA).any():        # reduce_max
                    flagged = True
    Q = _exact_q(x, label) if flagged else 0.0

    return np.float32((P + Q) / (N * (N - 1)))


_NC_CACHE: dict = {}


def kernel(output: np.ndarray, label: np.ndarray) -> np.ndarray:
    x = np.asarray(output, dtype=np.float32)
    lab = np.asarray(label).astype(np.int64)
    assert x.shape == (N, D) and lab.shape == (N,)

    screen = _screen(x.astype(np.float64))
    W = screen[2]
    in_maps = prep_inputs(x, lab, screen)
    if W not in _NC_CACHE:
        _NC_CACHE[W] = build_nc(widths=W)
    nc = _NC_CACHE[W]

    res = run_bass_kernel_spmd(nc, in_maps, core_ids=list(range(CORES)))
    loss = finish(res.results, _schedule(W), x, lab)
    return np.asarray(loss, dtype=np.float32)
